# revision 12
# baseline (speedup 1.0000x reference)
"""Multi-head attention (B=2, S=2048, E=1024, H=16, D=64) on 8 TRN2 NeuronCores.

Sharding: tensor-parallel over heads (2 heads/core) for QKV projections and
attention; an on-device AllToAll reshards so each core owns 512 output rows;
the output projection runs in a transposed layout (wo stationary, out [E, RL])
and the host un-transposes. Inputs host-cast to bf16, x host-transposed.

Structure (per core):
- Startup: x^T streams in as 8x 1MB chunks on both HWDGE queues; K(b0)+Q(b0)
  projections accumulate chunk-major in 8 PSUM banks so they finish with the
  DMA. V(b0)/b1 projections + V-transposes run later as PE filler.
- Attention is paced by the scalar engine (exp eviction), which has no clock
  throttle: per kc the two heads' score matmuls run in disjoint 64-row PE
  groups (concurrent), one ACTIVATE evicts both heads' scores with
  scale=1/8, and the previous unit's PV matmuls + projection fillers soak the
  PE slack. A ones-column appended to V accumulates the softmax denominator;
  normalization = DVE reciprocal + 1-row ones-matmul broadcast (PE) + DVE
  multiply. ACT-pacing keeps all 8 cores in lockstep so the AllToAll entry
  skew stays small.
- Out projection: i-major accumulation with wo chunks stationary over all 8
  PSUM banks; bias added as a per-partition scalar during eviction; dummy
  matmuls keep the PE clock-gate warm across the AllToAll window.
"""

import sys

if "/opt/trn_rl_repo" not in sys.path:
    sys.path.insert(0, "/opt/trn_rl_repo")

from contextlib import ExitStack

import numpy as np

import concourse.bacc as bacc
import concourse.mybir as mybir
import concourse.tile as tile
from concourse.masks import make_identity

F32 = mybir.dt.float32
BF16 = mybir.dt.bfloat16
AF = mybir.ActivationFunctionType

_CACHE = {}


def build_kernel(B=2, S=2048, E=1024, H=16, D=64, N_CORES=8):
    HL = H // N_CORES          # heads per core = 2
    HIDL = HL * D              # hidden dims per core = 128
    R = B * S                  # 4096
    RL = R // N_CORES          # 512 output rows per core
    EC = E // 128              # 8 x^T chunks
    S128 = S // 128            # 16 key chunks per batch
    QB = 512                   # q-block width
    NQB = S // QB              # 4 q-blocks per batch
    NG = R // QB               # 8 groups == N_CORES
    assert HIDL == 128 and D == 64 and QB == RL and NG == N_CORES

    nc = bacc.Bacc("TRN2", target_bir_lowering=False, debug=False,
                   num_devices=N_CORES)

    xt_d = nc.dram_tensor("xt", [E, R], BF16, kind="ExternalInput")
    wq_d = nc.dram_tensor("wq", [E, HIDL], BF16, kind="ExternalInput")
    wk_d = nc.dram_tensor("wk", [E, HIDL], BF16, kind="ExternalInput")
    wv_d = nc.dram_tensor("wv", [E, HIDL], BF16, kind="ExternalInput")
    wo_d = nc.dram_tensor("wo", [E, E], BF16, kind="ExternalInput")
    bq_d = nc.dram_tensor("bq", [HIDL, 1], F32, kind="ExternalInput")
    bk_d = nc.dram_tensor("bk", [HIDL, 1], F32, kind="ExternalInput")
    bv_d = nc.dram_tensor("bv", [HIDL, 1], F32, kind="ExternalInput")
    bo_d = nc.dram_tensor("bo", [128, EC], F32, kind="ExternalInput")
    out_d = nc.dram_tensor("out", [E, RL], F32, kind="ExternalOutput")

    with tile.TileContext(nc) as tc, ExitStack() as ctx:
        const = ctx.enter_context(tc.tile_pool(name="const", bufs=1))
        big = ctx.enter_context(tc.tile_pool(name="big", bufs=1))
        ebp = ctx.enter_context(tc.tile_pool(name="ebp", bufs=1))
        rp = ctx.enter_context(tc.tile_pool(name="rp", bufs=1))
        stage = ctx.enter_context(tc.tile_pool(name="stage", bufs=1))
        dram = ctx.enter_context(tc.tile_pool(name="dram", bufs=1, space="DRAM"))

        # ---- constants / small weights (sync queue first) ----
        ident = const.tile([128, 128], BF16)
        make_identity(nc, ident)
        ones_t = const.tile([128, 64], F32)
        nc.vector.memset(ones_t, 1.0)
        b_tiles = {}
        for bname, bd in (("bq", bq_d), ("bk", bk_d), ("bv", bv_d)):
            t = const.tile([HIDL, 1], F32, name=f"{bname}_sb")
            nc.sync.dma_start(out=t[:], in_=bd[:])
            b_tiles[bname] = t
        w_tiles = {}
        for wname, wd in (("wq", wq_d), ("wk", wk_d), ("wv", wv_d)):
            for i in range(EC):
                t = const.tile([128, HIDL], BF16, name=f"{wname}_{i}")
                nc.sync.dma_start(out=t[:], in_=wd[128 * i:128 * (i + 1), :])
                w_tiles[(wname, i)] = t

        # ---- x^T chunks split across both hwdge queues ----
        xT = big.tile([128, EC, R], BF16)
        for i in range(EC):
            eng = nc.scalar if i % 2 == 0 else nc.sync
            eng.dma_start(out=xT[:, i, :], in_=xt_d[128 * i:128 * (i + 1), :])

        # wo / bo needed only after the A2A; scalar queue, after xT
        bo_sb = const.tile([128, EC], F32)
        nc.scalar.dma_start(out=bo_sb[:], in_=bo_d[:])
        wo_tiles = []
        for i in range(EC):
            t = const.tile([128, E], BF16, name=f"wo_{i}")
            nc.scalar.dma_start(out=t[:], in_=wo_d[128 * i:128 * (i + 1), :])
            wo_tiles.append(t)

        QT = big.tile([128, R], BF16)
        KT = big.tile([128, R], BF16)
        VT = big.tile([128, R], BF16)
        Vext = big.tile([128, HL, B * S128, D + 1], BF16)
        nc.vector.memset(Vext[:, :, :, D:D + 1], 1.0)
        # per-head halves of the attention output (both live on partitions
        # 0-63; the a2a DMA stacks them into the shard's 128 hid rows)
        ATnA = big.tile([64, NG, QB], BF16)
        ATnB = big.tile([64, NG, QB], BF16)

        a2a_in = dram.tile([NG * HIDL, QB], BF16)
        a2a_out = dram.tile([NG * HIDL, QB], BF16)

        # ---- pass 1: K(b0) + Q(b0), chunk-major, overlapped with x^T DMA ----
        streams = [("wk", "bk", KT, rb) for rb in range(NQB)] + \
                  [("wq", "bq", QT, rb) for rb in range(NQB)]
        with tc.tile_pool(name="p1_psum", bufs=1, space="PSUM") as p1:
            p1_tiles = [p1.tile([128, QB], F32, tag=f"p{s}", name=f"p1_{s}")
                        for s in range(8)]
            for i in range(EC):
                for s, (wname, _, _, rb) in enumerate(streams):
                    nc.tensor.matmul(p1_tiles[s][:],
                                     w_tiles[(wname, i)][:],
                                     xT[:, i, QB * rb:QB * (rb + 1)],
                                     start=(i == 0), stop=(i == EC - 1))
            for s, (_, bname, out_t, rb) in enumerate(streams):
                nc.vector.tensor_scalar_add(
                    out=out_t[:, QB * rb:QB * (rb + 1)], in0=p1_tiles[s][:],
                    scalar1=b_tiles[bname][:])

        # ---- attention pools ----
        att_stack = ExitStack()
        att_psum = att_stack.enter_context(
            tc.tile_pool(name="att_psum", bufs=1, space="PSUM"))

        # filler work: one callable == one bite-sized chunk of PE work.
        # V projection streams also emit the DMA-transposes that build Vext
        # (xbar transpose -- no PE or PSUM involvement).
        def proj_stream(wname, bname, out_t, rb):
            def run():
                ps = att_psum.tile([128, QB], F32, tag="fill", bufs=2,
                                   name="fill_ps")
                for i in range(EC):
                    nc.tensor.matmul(ps[:], w_tiles[(wname, i)][:],
                                     xT[:, i, QB * rb:QB * (rb + 1)],
                                     start=(i == 0), stop=(i == EC - 1))
                nc.vector.tensor_scalar_add(
                    out=out_t[:, QB * rb:QB * (rb + 1)], in0=ps[:],
                    scalar1=b_tiles[bname][:])
                if wname == "wv":
                    for kt in range(4 * rb, 4 * rb + 4):
                        stg = rp.tile([128, 128], BF16, tag="vtr", bufs=2,
                                      name="vtr_stg")
                        nc.scalar.dma_start_transpose(
                            stg[:], VT[:, 128 * kt:128 * (kt + 1)])
                        for hl in range(HL):
                            nc.vector.tensor_copy(
                                out=Vext[:, hl, kt, 0:D],
                                in_=stg[:, D * hl:D * (hl + 1)])
            return run

        fillers = {
            0: [proj_stream("wv", "bv", VT, rb) for rb in range(NQB)],
            1: [proj_stream("wk", "bk", KT, rb) for rb in range(NQB, NQB + 3)],
            2: [proj_stream("wk", "bk", KT, NQB + 3),
                proj_stream("wq", "bq", QT, NQB),
                proj_stream("wq", "bq", QT, NQB + 1)],
            3: [proj_stream("wq", "bq", QT, NQB + 2),
                proj_stream("wq", "bq", QT, NQB + 3),
                proj_stream("wv", "bv", VT, NQB)],
            4: [proj_stream("wv", "bv", VT, rb)
                for rb in (NQB + 1, NQB + 2, NQB + 3)],
        }

        # ---- attention: ACT-paced pipeline over 8 (b, qb) units ----
        def norm_and_ship(pvts, g):
            # pvts[h] rows 0-63 = V.T@E, row 64 = softmax denominator.
            # DVE reads at most one PSUM operand per op: evict pvT to SBUF,
            # reciprocal there, PE-broadcast the row, multiply SBUF x PSUM.
            for h, atn in ((0, ATnA), (1, ATnB)):
                sb_pv = rp.tile([D + 1, QB], F32, tag="sbpv", bufs=2,
                                name="sb_pv")
                nc.vector.tensor_copy(out=sb_pv[:], in_=pvts[h][0:D + 1, :])
                r = rp.tile([128, QB], F32, tag="r", bufs=2, name="r_row")
                nc.vector.reciprocal(r[64:65, :], sb_pv[64:65, :])
                bc = att_psum.tile([128, QB], F32, tag="fill", bufs=2,
                                   name="bc_ps")
                nc.tensor.matmul(bc[0:64, :], ones_t[64:65, :], r[64:65, :],
                                 start=True, stop=True)
                nc.vector.tensor_mul(out=atn[:, g, :], in0=sb_pv[0:64, :],
                                     in1=bc[0:64, :])
            nc.sync.dma_start(out=a2a_in[HIDL * g:HIDL * g + 64, :],
                              in_=ATnA[:, g, :])
            nc.sync.dma_start(out=a2a_in[HIDL * g + 64:HIDL * (g + 1), :],
                              in_=ATnB[:, g, :])

        units = [(b, qb) for b in range(B) for qb in range(NQB)]
        pending_norm = None  # (pvts, g) normed early in the following unit
        for u, (b, qb) in enumerate(units):
            q0 = b * S + QB * qb
            # PV of kc trails its exp by pv_lag periods so exp-gated matmuls
            # never block the scores stream; unit 0 trails further to give
            # the V(b0) fillers time to build Vext.
            pv_lag = 6 if u == 0 else 2
            ebs = []
            fill = list(fillers.get(u, []))
            pvts = [att_psum.tile([128, QB], F32, tag="pv", bufs=2,
                                  name=f"pv{h}") for h in range(HL)]

            def pv_kc(kc):
                for h in range(HL):
                    nc.tensor.matmul(
                        pvts[h][0:D + 1, :],
                        Vext[:, h, b * S128 + kc, :],
                        ebs[kc][:, h, :],
                        start=(kc == 0), stop=(kc == S128 - 1))

            for kc in range(S128):
                ps = att_psum.tile([128, HL, QB], F32, tag="sc", bufs=2,
                                   name="sc_ps")
                for h in range(HL):
                    hs = slice(64 * h, 64 * (h + 1))
                    nc.tensor.matmul(
                        ps[:, h, :],
                        KT[hs, b * S + 128 * kc:b * S + 128 * (kc + 1)],
                        QT[hs, q0:q0 + QB], start=True, stop=True)
                eb = ebp.tile([128, HL, QB], BF16, tag="eb", bufs=10,
                              name="eb")
                nc.scalar.activation(eb[:], ps[:], AF.Exp, scale=0.125)
                ebs.append(eb)
                if kc == 1 and pending_norm is not None:
                    # must precede this unit's first PV write, which recycles
                    # the pv slots the pending norm still reads
                    norm_and_ship(*pending_norm)
                    pending_norm = None
                if kc >= pv_lag:
                    pv_kc(kc - pv_lag)
                if fill:
                    fill.pop(0)()
            for kc in range(S128 - pv_lag, S128):
                pv_kc(kc)
            pending_norm = (pvts, u)
        norm_and_ship(*pending_norm)

        nc.gpsimd.collective_compute(
            "AllToAll", mybir.AluOpType.bypass,
            replica_groups=[list(range(N_CORES))],
            ins=[a2a_in.opt()], outs=[a2a_out.opt()])

        # keep the PE clock-gate warm across the AllToAll window; the moving
        # operand is copied from the last attention output so these matmuls
        # are scheduled after attention, inside the A2A wait
        wup_sink = dram.tile([1, 4], BF16)
        wup_sb = stage.tile([1, 4], BF16, tag="wup")
        wup_mv = const.tile([128, 128], BF16)
        nc.vector.memset(wup_mv[:], 1.0)
        nc.vector.tensor_copy(out=wup_mv[0:64, :], in_=ATnB[:, NG - 1, 0:128])
        for _ in range(28):
            wps = att_psum.tile([128, QB], F32, tag="fill", bufs=2,
                                name="wup_ps")
            for w in range(8):
                nc.tensor.matmul(wps[:, 0:128], ident[:], wup_mv[:],
                                 start=(w == 0), stop=(w == 7))
            nc.vector.tensor_copy(out=wup_sb[:], in_=wps[0:1, 0:4])
        nc.sync.dma_start(out=wup_sink[:], in_=wup_sb[:])
        att_stack.close()

        # ---- out projection (transposed: out[e, q] = wo.T-chunks @ A) ----
        AT = big.tile([128, EC, QB], BF16)
        for i in range(EC):
            eng = nc.scalar if i % 2 == 0 else nc.sync
            eng.dma_start(out=AT[:, i, :],
                          in_=a2a_out[HIDL * i:HIDL * (i + 1), :])
        with tc.tile_pool(name="op_psum", bufs=1, space="PSUM") as opp:
            pso = opp.tile([128, EC, QB], F32, name="pso")
            for i in range(EC):
                for e in range(EC):
                    nc.tensor.matmul(pso[:, e, :],
                                     wo_tiles[i][:, 128 * e:128 * (e + 1)],
                                     AT[:, i, :],
                                     start=(i == 0), stop=(i == EC - 1))
            for e in range(EC):
                o_sb = stage.tile([128, QB], F32, tag="osb", bufs=2)
                nc.vector.tensor_scalar_add(out=o_sb[:], in0=pso[:, e, :],
                                            scalar1=bo_sb[:, e:e + 1])
                eng = nc.scalar if e % 2 == 0 else nc.sync
                eng.dma_start(out=out_d[128 * e:128 * (e + 1), :], in_=o_sb[:])

    nc.compile()
    return nc


def shard_inputs(x, Wq, bq, Wk, bk, Wv, bv, Wo, bo, N_CORES=8):
    """Host-side sharding: full fp32 inputs -> per-core in_maps."""
    import ml_dtypes
    bf16 = ml_dtypes.bfloat16
    B, S, E = x.shape
    R = B * S
    HIDL = E // N_CORES
    xt = np.ascontiguousarray(x.reshape(R, E).T).astype(bf16)
    wo = np.ascontiguousarray(Wo).astype(bf16)
    bo_b = np.ascontiguousarray(bo.reshape(E // 128, 128).T).astype(np.float32)
    in_maps = []
    for c in range(N_CORES):
        cs = slice(HIDL * c, HIDL * (c + 1))
        in_maps.append({
            "xt": xt,
            "wq": np.ascontiguousarray(Wq[:, cs]).astype(bf16),
            "wk": np.ascontiguousarray(Wk[:, cs]).astype(bf16),
            "wv": np.ascontiguousarray(Wv[:, cs]).astype(bf16),
            "wo": wo,
            "bq": np.ascontiguousarray(bq[cs].reshape(HIDL, 1)).astype(np.float32),
            "bk": np.ascontiguousarray(bk[cs].reshape(HIDL, 1)).astype(np.float32),
            "bv": np.ascontiguousarray(bv[cs].reshape(HIDL, 1)).astype(np.float32),
            "bo": bo_b,
        })
    return in_maps


def assemble(results, N_CORES=8):
    """Per-core out [E, RL] (core c = q rows 512c..512c+512) -> [B, S, E]."""
    full = np.concatenate([results[i]["out"] for i in range(N_CORES)], axis=1)
    return np.ascontiguousarray(full.T).reshape(2, 2048, 1024)


def kernel(x, Wq, bq, Wk, bk, Wv, bv, Wo, bo):
    from concourse.bass_utils import run_bass_kernel_spmd

    args = [np.asarray(a, dtype=np.float32) for a in
            (x, Wq, bq, Wk, bk, Wv, bv, Wo, bo)]
    if "nc" not in _CACHE:
        _CACHE["nc"] = build_kernel()
    nc = _CACHE["nc"]
    in_maps = shard_inputs(*args)
    res = run_bass_kernel_spmd(nc, in_maps, core_ids=list(range(8)))
    return assemble(res.results)


# revision 17
# speedup vs baseline: 1.1887x; 1.1887x over previous
"""Multi-head attention (B=2, S=2048, E=1024, H=16, D=64) on 8 TRN2 NeuronCores.

Sharding: tensor-parallel over heads (2 heads/core) for QKV projections and
attention; an on-device AllToAll reshards so each core owns 512 output rows;
the output projection runs in a transposed layout (wo stationary, out [E, RL])
and the host un-transposes. Inputs host-cast to bf16, x host-transposed.

Structure (per core):
- Startup: x^T streams in as 8x 1MB chunks on both HWDGE queues; K(b0)+Q(b0)
  projections accumulate chunk-major in 8 PSUM banks so they finish with the
  DMA. V(b0)/b1 projections + V-transposes run later as PE filler.
- Attention is paced by the scalar engine (exp eviction), which has no clock
  throttle: per kc the two heads' score matmuls run in disjoint 64-row PE
  groups (concurrent), one ACTIVATE evicts both heads' scores with
  scale=1/8, and the previous unit's PV matmuls + projection fillers soak the
  PE slack. A ones-column appended to V accumulates the softmax denominator;
  normalization = DVE reciprocal + 1-row ones-matmul broadcast (PE) + DVE
  multiply. ACT-pacing keeps all 8 cores in lockstep so the AllToAll entry
  skew stays small.
- Out projection: i-major accumulation with wo chunks stationary over all 8
  PSUM banks; bias added as a per-partition scalar during eviction; dummy
  matmuls keep the PE clock-gate warm across the AllToAll window.
"""

import sys

if "/opt/trn_rl_repo" not in sys.path:
    sys.path.insert(0, "/opt/trn_rl_repo")

from contextlib import ExitStack

import numpy as np

import concourse.bacc as bacc
import concourse.mybir as mybir
import concourse.tile as tile
from concourse.masks import make_identity

F32 = mybir.dt.float32
BF16 = mybir.dt.bfloat16
AF = mybir.ActivationFunctionType

_CACHE = {}


def build_kernel(B=2, S=2048, E=1024, H=16, D=64, N_CORES=8):
    HL = H // N_CORES          # heads per core = 2
    HIDL = HL * D              # hidden dims per core = 128
    R = B * S                  # 4096
    RL = R // N_CORES          # 512 output rows per core
    EC = E // 128              # 8 x^T chunks
    S128 = S // 128            # 16 key chunks per batch
    QB = 512                   # q-block width
    NQB = S // QB              # 4 q-blocks per batch
    NG = R // QB               # 8 groups == N_CORES
    assert HIDL == 128 and D == 64 and QB == RL and NG == N_CORES

    nc = bacc.Bacc("TRN2", target_bir_lowering=False, debug=False,
                   num_devices=N_CORES)

    xt_d = nc.dram_tensor("xt", [E, R], BF16, kind="ExternalInput")
    wq_d = nc.dram_tensor("wq", [E, HIDL], BF16, kind="ExternalInput")
    wk_d = nc.dram_tensor("wk", [E, HIDL], BF16, kind="ExternalInput")
    wv_d = nc.dram_tensor("wv", [E, HIDL], BF16, kind="ExternalInput")
    wo_d = nc.dram_tensor("wo", [E, E], BF16, kind="ExternalInput")
    bq_d = nc.dram_tensor("bq", [HIDL, 1], F32, kind="ExternalInput")
    bk_d = nc.dram_tensor("bk", [HIDL, 1], F32, kind="ExternalInput")
    bv_d = nc.dram_tensor("bv", [HIDL, 1], F32, kind="ExternalInput")
    bo_d = nc.dram_tensor("bo", [128, EC], F32, kind="ExternalInput")
    out_d = nc.dram_tensor("out", [E, RL], F32, kind="ExternalOutput")

    with tile.TileContext(nc) as tc, ExitStack() as ctx:
        const = ctx.enter_context(tc.tile_pool(name="const", bufs=1))
        big = ctx.enter_context(tc.tile_pool(name="big", bufs=1))
        ebp = ctx.enter_context(tc.tile_pool(name="ebp", bufs=1))
        rp = ctx.enter_context(tc.tile_pool(name="rp", bufs=1))
        stage = ctx.enter_context(tc.tile_pool(name="stage", bufs=1))
        dram = ctx.enter_context(tc.tile_pool(name="dram", bufs=1, space="DRAM"))

        # ---- constants / small weights (sync queue first) ----
        ident = const.tile([128, 128], BF16)
        make_identity(nc, ident)
        ones_t = const.tile([128, 64], BF16)
        nc.vector.memset(ones_t, 1.0)
        b_tiles = {}
        for bname, bd in (("bq", bq_d), ("bk", bk_d), ("bv", bv_d)):
            t = const.tile([HIDL, 1], F32, name=f"{bname}_sb")
            nc.sync.dma_start(out=t[:], in_=bd[:])
            b_tiles[bname] = t
        w_tiles = {}
        for wname, wd in (("wq", wq_d), ("wk", wk_d), ("wv", wv_d)):
            for i in range(EC):
                t = const.tile([128, HIDL], BF16, name=f"{wname}_{i}")
                nc.sync.dma_start(out=t[:], in_=wd[128 * i:128 * (i + 1), :])
                w_tiles[(wname, i)] = t

        # ---- x^T chunks split across both hwdge queues ----
        xT = big.tile([128, EC, R], BF16)
        for i in range(EC):
            eng = nc.scalar if i % 2 == 0 else nc.sync
            eng.dma_start(out=xT[:, i, :], in_=xt_d[128 * i:128 * (i + 1), :])

        # wo / bo needed only after the A2A; scalar queue, after xT
        bo_sb = const.tile([128, EC], F32)
        nc.scalar.dma_start(out=bo_sb[:], in_=bo_d[:])
        wo_tiles = []
        for i in range(EC):
            t = const.tile([128, E], BF16, name=f"wo_{i}")
            nc.scalar.dma_start(out=t[:], in_=wo_d[128 * i:128 * (i + 1), :])
            wo_tiles.append(t)

        QT = big.tile([128, R], BF16)
        KT = big.tile([128, R], BF16)
        VT = big.tile([128, R], BF16)
        Vext = big.tile([128, HL, B * S128, D + 1], BF16)
        nc.vector.memset(Vext[:, :, :, D:D + 1], 1.0)
        # per-head halves of the attention output (both live on partitions
        # 0-63; the a2a DMA stacks them into the shard's 128 hid rows)
        ATnA = big.tile([64, NG, QB], BF16)
        ATnB = big.tile([64, NG, QB], BF16)

        a2a_in = dram.tile([NG * HIDL, QB], BF16)
        a2a_out = dram.tile([NG * HIDL, QB], BF16)

        # ---- pass 1: K(b0) + Q(b0), chunk-major, overlapped with x^T DMA ----
        streams = [("wk", "bk", KT, rb) for rb in range(NQB)] + \
                  [("wq", "bq", QT, rb) for rb in range(NQB)]
        with tc.tile_pool(name="p1_psum", bufs=1, space="PSUM") as p1:
            p1_tiles = [p1.tile([128, QB], F32, tag=f"p{s}", name=f"p1_{s}")
                        for s in range(8)]
            for i in range(EC):
                for s, (wname, _, _, rb) in enumerate(streams):
                    nc.tensor.matmul(p1_tiles[s][:],
                                     w_tiles[(wname, i)][:],
                                     xT[:, i, QB * rb:QB * (rb + 1)],
                                     start=(i == 0), stop=(i == EC - 1))
            for s, (_, bname, out_t, rb) in enumerate(streams):
                nc.vector.tensor_scalar_add(
                    out=out_t[:, QB * rb:QB * (rb + 1)], in0=p1_tiles[s][:],
                    scalar1=b_tiles[bname][:])

        # ---- attention pools ----
        att_stack = ExitStack()
        att_psum = att_stack.enter_context(
            tc.tile_pool(name="att_psum", bufs=1, space="PSUM"))

        # filler work: one callable == one bite-sized chunk of PE work.
        # V projection streams also emit the DMA-transposes that build Vext
        # (xbar transpose -- no PE or PSUM involvement).
        def proj_stream(wname, bname, out_t, rb):
            def run():
                ps = att_psum.tile([128, QB], F32, tag="fill", bufs=2,
                                   name="fill_ps")
                for i in range(EC):
                    nc.tensor.matmul(ps[:], w_tiles[(wname, i)][:],
                                     xT[:, i, QB * rb:QB * (rb + 1)],
                                     start=(i == 0), stop=(i == EC - 1))
                nc.vector.tensor_scalar_add(
                    out=out_t[:, QB * rb:QB * (rb + 1)], in0=ps[:],
                    scalar1=b_tiles[bname][:])
                if wname == "wv":
                    for kt in range(4 * rb, 4 * rb + 4):
                        stg = rp.tile([128, 128], BF16, tag="vtr", bufs=2,
                                      name="vtr_stg")
                        nc.scalar.dma_start_transpose(
                            stg[:], VT[:, 128 * kt:128 * (kt + 1)])
                        for hl in range(HL):
                            nc.vector.tensor_copy(
                                out=Vext[:, hl, kt, 0:D],
                                in_=stg[:, D * hl:D * (hl + 1)])
            return run

        fillers = {
            0: [proj_stream("wv", "bv", VT, rb) for rb in range(NQB)],
            1: [proj_stream("wk", "bk", KT, rb) for rb in range(NQB, NQB + 3)],
            2: [proj_stream("wk", "bk", KT, NQB + 3),
                proj_stream("wq", "bq", QT, NQB),
                proj_stream("wq", "bq", QT, NQB + 1)],
            3: [proj_stream("wq", "bq", QT, NQB + 2),
                proj_stream("wq", "bq", QT, NQB + 3),
                proj_stream("wv", "bv", VT, NQB)],
            4: [proj_stream("wv", "bv", VT, rb)
                for rb in (NQB + 1, NQB + 2, NQB + 3)],
        }

        # ---- attention: ACT-paced pipeline over 8 (b, qb) units ----
        # pvts[h] rows 0-63 = V.T@E, row 64 = softmax denominator. The norm
        # is split: the PSUM->SBUF eviction runs at the next unit's kc==0
        # (so the pv slots recycle immediately), the reciprocal runs async on
        # DVE, and only a cheap bf16 1-row broadcast matmul + multiply touch
        # the PE stream at kc==6, when the reciprocal is long done.
        def norm_copy(pvts):
            sbs = []
            for h in range(HL):
                sb_pv = rp.tile([D, QB], F32, tag="sbpv", bufs=2,
                                name="sb_pv")
                nc.vector.tensor_copy(out=sb_pv[:], in_=pvts[h][0:D, :])
                # stage the denominator row to partition 0: the custom-DVE
                # reciprocal mis-addresses partition-offset slices
                den = rp.tile([1, QB], F32, tag="den", bufs=2, name="den")
                nc.vector.tensor_copy(out=den[:], in_=pvts[h][D:D + 1, :])
                r = rp.tile([1, QB], F32, tag="r", bufs=2, name="r_row")
                nc.vector.reciprocal_approx_fast(r[:], den[:])
                rb = rp.tile([1, QB], BF16, tag="rb", bufs=2, name="rb_row")
                nc.vector.tensor_copy(out=rb[:], in_=r[:])
                sbs.append((sb_pv, rb))
            return sbs

        def norm_finish(sbs, g):
            for h, atn in ((0, ATnA), (1, ATnB)):
                sb_pv, rb = sbs[h]
                bc = att_psum.tile([128, QB], F32, tag="fill", bufs=2,
                                   name="bc_ps")
                nc.tensor.matmul(bc[0:64, :], ones_t[0:1, :], rb[0:1, :],
                                 start=True, stop=True)
                nc.vector.tensor_mul(out=atn[:, g, :], in0=sb_pv[:],
                                     in1=bc[0:64, :])
            nc.sync.dma_start(out=a2a_in[HIDL * g:HIDL * g + 64, :],
                              in_=ATnA[:, g, :])
            nc.sync.dma_start(out=a2a_in[HIDL * g + 64:HIDL * (g + 1), :],
                              in_=ATnB[:, g, :])

        units = [(b, qb) for b in range(B) for qb in range(NQB)]
        pending_norm = None  # (pvts, g) normed early in the following unit
        for u, (b, qb) in enumerate(units):
            q0 = b * S + QB * qb
            # PV of kc trails its exp by pv_lag periods so exp-gated matmuls
            # never block the scores stream; unit 0 trails further to give
            # the V(b0) fillers time to build Vext.
            pv_lag = 6 if u == 0 else 2
            ebs = []
            fill = list(fillers.get(u, []))
            pvts = [att_psum.tile([128, QB], F32, tag="pv", bufs=2,
                                  name=f"pv{h}") for h in range(HL)]

            def pv_kc(kc):
                for h in range(HL):
                    nc.tensor.matmul(
                        pvts[h][0:D + 1, :],
                        Vext[:, h, b * S128 + kc, :],
                        ebs[kc][:, h, :],
                        start=(kc == 0), stop=(kc == S128 - 1))

            for kc in range(S128):
                ps = att_psum.tile([128, HL, QB], F32, tag="sc", bufs=2,
                                   name="sc_ps")
                for h in range(HL):
                    hs = slice(64 * h, 64 * (h + 1))
                    nc.tensor.matmul(
                        ps[:, h, :],
                        KT[hs, b * S + 128 * kc:b * S + 128 * (kc + 1)],
                        QT[hs, q0:q0 + QB], start=True, stop=True)
                eb = ebp.tile([128, HL, QB], BF16, tag="eb", bufs=10,
                              name="eb")
                nc.scalar.activation(eb[:], ps[:], AF.Exp, scale=0.125)
                ebs.append(eb)
                if kc == 0 and pending_norm is not None:
                    # the copies must precede this unit's first PV write,
                    # which recycles the pv slots the norm still reads
                    pending_norm = (norm_copy(pending_norm[0]),
                                    pending_norm[1])
                if kc == 6 and pending_norm is not None:
                    norm_finish(*pending_norm)
                    pending_norm = None
                if kc >= pv_lag:
                    pv_kc(kc - pv_lag)
                if fill:
                    fill.pop(0)()
            for kc in range(S128 - pv_lag, S128):
                pv_kc(kc)
            pending_norm = (pvts, u)
        norm_finish(norm_copy(pending_norm[0]), pending_norm[1])

        nc.gpsimd.collective_compute(
            "AllToAll", mybir.AluOpType.bypass,
            replica_groups=[list(range(N_CORES))],
            ins=[a2a_in.opt()], outs=[a2a_out.opt()])

        # keep the PE clock-gate warm across the AllToAll window; the moving
        # operand is copied from the last attention output so these matmuls
        # are scheduled after attention, inside the A2A wait
        wup_sink = dram.tile([1, 4], BF16)
        wup_sb = stage.tile([1, 4], BF16, tag="wup")
        wup_mv = const.tile([128, 128], BF16)
        nc.vector.memset(wup_mv[:], 1.0)
        nc.vector.tensor_copy(out=wup_mv[0:64, :], in_=ATnB[:, NG - 1, 0:128])
        for _ in range(28):
            wps = att_psum.tile([128, QB], F32, tag="fill", bufs=2,
                                name="wup_ps")
            for w in range(8):
                nc.tensor.matmul(wps[:, 0:128], ident[:], wup_mv[:],
                                 start=(w == 0), stop=(w == 7))
            nc.vector.tensor_copy(out=wup_sb[:], in_=wps[0:1, 0:4])
        nc.sync.dma_start(out=wup_sink[:], in_=wup_sb[:])
        att_stack.close()

        # ---- out projection (transposed: out[e, q] = wo.T-chunks @ A) ----
        AT = big.tile([128, EC, QB], BF16)
        for i in range(EC):
            eng = nc.scalar if i % 2 == 0 else nc.sync
            eng.dma_start(out=AT[:, i, :],
                          in_=a2a_out[HIDL * i:HIDL * (i + 1), :])
        with tc.tile_pool(name="op_psum", bufs=1, space="PSUM") as opp:
            pso = opp.tile([128, EC, QB], F32, name="pso")
            for i in range(EC):
                for e in range(EC):
                    nc.tensor.matmul(pso[:, e, :],
                                     wo_tiles[i][:, 128 * e:128 * (e + 1)],
                                     AT[:, i, :],
                                     start=(i == 0), stop=(i == EC - 1))
            for e in range(EC):
                o_sb = stage.tile([128, QB], F32, tag="osb", bufs=2)
                nc.vector.tensor_scalar_add(out=o_sb[:], in0=pso[:, e, :],
                                            scalar1=bo_sb[:, e:e + 1])
                eng = nc.scalar if e % 2 == 0 else nc.sync
                eng.dma_start(out=out_d[128 * e:128 * (e + 1), :], in_=o_sb[:])

    nc.compile()
    return nc


def shard_inputs(x, Wq, bq, Wk, bk, Wv, bv, Wo, bo, N_CORES=8):
    """Host-side sharding: full fp32 inputs -> per-core in_maps."""
    import ml_dtypes
    bf16 = ml_dtypes.bfloat16
    B, S, E = x.shape
    R = B * S
    HIDL = E // N_CORES
    xt = np.ascontiguousarray(x.reshape(R, E).T).astype(bf16)
    wo = np.ascontiguousarray(Wo).astype(bf16)
    bo_b = np.ascontiguousarray(bo.reshape(E // 128, 128).T).astype(np.float32)
    in_maps = []
    for c in range(N_CORES):
        cs = slice(HIDL * c, HIDL * (c + 1))
        in_maps.append({
            "xt": xt,
            "wq": np.ascontiguousarray(Wq[:, cs]).astype(bf16),
            "wk": np.ascontiguousarray(Wk[:, cs]).astype(bf16),
            "wv": np.ascontiguousarray(Wv[:, cs]).astype(bf16),
            "wo": wo,
            "bq": np.ascontiguousarray(bq[cs].reshape(HIDL, 1)).astype(np.float32),
            "bk": np.ascontiguousarray(bk[cs].reshape(HIDL, 1)).astype(np.float32),
            "bv": np.ascontiguousarray(bv[cs].reshape(HIDL, 1)).astype(np.float32),
            "bo": bo_b,
        })
    return in_maps


def assemble(results, N_CORES=8):
    """Per-core out [E, RL] (core c = q rows 512c..512c+512) -> [B, S, E]."""
    full = np.concatenate([results[i]["out"] for i in range(N_CORES)], axis=1)
    return np.ascontiguousarray(full.T).reshape(2, 2048, 1024)


def kernel(x, Wq, bq, Wk, bk, Wv, bv, Wo, bo):
    from concourse.bass_utils import run_bass_kernel_spmd

    args = [np.asarray(a, dtype=np.float32) for a in
            (x, Wq, bq, Wk, bk, Wv, bv, Wo, bo)]
    if "nc" not in _CACHE:
        _CACHE["nc"] = build_kernel()
    nc = _CACHE["nc"]
    in_maps = shard_inputs(*args)
    res = run_bass_kernel_spmd(nc, in_maps, core_ids=list(range(8)))
    return assemble(res.results)


# revision 22
# speedup vs baseline: 1.1974x; 1.0073x over previous
"""Multi-head attention (B=2, S=2048, E=1024, H=16, D=64) on 8 TRN2 NeuronCores.

Sharding: tensor-parallel over heads (2 heads/core) for QKV projections and
attention; an on-device AllToAll reshards so each core owns 512 output rows;
the output projection runs in a transposed layout (wo stationary, out [E, RL])
and the host un-transposes. Inputs host-cast to bf16, x host-transposed.

Structure (per core):
- Startup: x^T streams in as 8x 1MB chunks on both HWDGE queues; K(b0)+Q(b0)
  projections accumulate chunk-major in 8 PSUM banks so they finish with the
  DMA. V(b0)/b1 projections + V-transposes run later as PE filler.
- Attention is paced by the scalar engine (exp eviction), which has no clock
  throttle: per kc the two heads' score matmuls run in disjoint 64-row PE
  groups (concurrent), one ACTIVATE evicts both heads' scores with
  scale=1/8, and the previous unit's PV matmuls + projection fillers soak the
  PE slack. A ones-column appended to V accumulates the softmax denominator;
  normalization = DVE reciprocal + 1-row ones-matmul broadcast (PE) + DVE
  multiply. ACT-pacing keeps all 8 cores in lockstep so the AllToAll entry
  skew stays small.
- Out projection: i-major accumulation with wo chunks stationary over all 8
  PSUM banks; bias added as a per-partition scalar during eviction; dummy
  matmuls keep the PE clock-gate warm across the AllToAll window.
"""

import sys

if "/opt/trn_rl_repo" not in sys.path:
    sys.path.insert(0, "/opt/trn_rl_repo")

from contextlib import ExitStack

import numpy as np

import concourse.bacc as bacc
import concourse.mybir as mybir
import concourse.tile as tile
from concourse.masks import make_identity

F32 = mybir.dt.float32
BF16 = mybir.dt.bfloat16
AF = mybir.ActivationFunctionType

_CACHE = {}


def build_kernel(B=2, S=2048, E=1024, H=16, D=64, N_CORES=8):
    HL = H // N_CORES          # heads per core = 2
    HIDL = HL * D              # hidden dims per core = 128
    R = B * S                  # 4096
    RL = R // N_CORES          # 512 output rows per core
    EC = E // 128              # 8 x^T chunks
    S128 = S // 128            # 16 key chunks per batch
    QB = 512                   # q-block width
    NQB = S // QB              # 4 q-blocks per batch
    NG = R // QB               # 8 groups == N_CORES
    assert HIDL == 128 and D == 64 and QB == RL and NG == N_CORES

    nc = bacc.Bacc("TRN2", target_bir_lowering=False, debug=False,
                   num_devices=N_CORES)

    xt_d = nc.dram_tensor("xt", [E, R], BF16, kind="ExternalInput")
    wq_d = nc.dram_tensor("wq", [E, HIDL], BF16, kind="ExternalInput")
    wk_d = nc.dram_tensor("wk", [E, HIDL], BF16, kind="ExternalInput")
    wv_d = nc.dram_tensor("wv", [E, HIDL], BF16, kind="ExternalInput")
    wo_d = nc.dram_tensor("wo", [E, E], BF16, kind="ExternalInput")
    bq_d = nc.dram_tensor("bq", [HIDL, 1], F32, kind="ExternalInput")
    bk_d = nc.dram_tensor("bk", [HIDL, 1], F32, kind="ExternalInput")
    bv_d = nc.dram_tensor("bv", [HIDL, 1], F32, kind="ExternalInput")
    bo_d = nc.dram_tensor("bo", [128, EC], F32, kind="ExternalInput")
    out_d = nc.dram_tensor("out", [E, RL], F32, kind="ExternalOutput")

    with tile.TileContext(nc) as tc, ExitStack() as ctx:
        const = ctx.enter_context(tc.tile_pool(name="const", bufs=1))
        big = ctx.enter_context(tc.tile_pool(name="big", bufs=1))
        ebp = ctx.enter_context(tc.tile_pool(name="ebp", bufs=1))
        rp = ctx.enter_context(tc.tile_pool(name="rp", bufs=1))
        stage = ctx.enter_context(tc.tile_pool(name="stage", bufs=1))
        dram = ctx.enter_context(tc.tile_pool(name="dram", bufs=1, space="DRAM"))

        # ---- constants / small weights (sync queue first) ----
        ident = const.tile([128, 128], BF16)
        make_identity(nc, ident)
        ones_t = const.tile([128, 64], BF16)
        nc.vector.memset(ones_t, 1.0)
        b_tiles = {}
        for bname, bd in (("bq", bq_d), ("bk", bk_d), ("bv", bv_d)):
            t = const.tile([HIDL, 1], F32, name=f"{bname}_sb")
            nc.scalar.dma_start(out=t[:], in_=bd[:])
            b_tiles[bname] = t
        w_tiles = {}
        for wname, wd in (("wq", wq_d), ("wk", wk_d), ("wv", wv_d)):
            for i in range(EC):
                t = const.tile([128, HIDL], BF16, name=f"{wname}_{i}")
                nc.scalar.dma_start(out=t[:],
                                    in_=wd[128 * i:128 * (i + 1), :])
                w_tiles[(wname, i)] = t

        # ---- x^T chunks back-to-back on the sync queue (weights ride the
        # scalar queue so the two never compete for the same queue slot) ----
        xT = big.tile([128, EC, R], BF16)
        for i in range(EC):
            nc.sync.dma_start(out=xT[:, i, :],
                              in_=xt_d[128 * i:128 * (i + 1), :])

        # wo / bo needed only after the A2A; scalar queue, after xT
        bo_sb = const.tile([128, EC], F32)
        nc.scalar.dma_start(out=bo_sb[:], in_=bo_d[:])
        wo_tiles = []
        for i in range(EC):
            t = const.tile([128, E], BF16, name=f"wo_{i}")
            nc.scalar.dma_start(out=t[:], in_=wo_d[128 * i:128 * (i + 1), :])
            wo_tiles.append(t)

        QT = big.tile([128, R], BF16)
        KT = big.tile([128, R], BF16)
        VT = big.tile([128, R], BF16)
        Vext = big.tile([128, HL, B * S128, D + 1], BF16)
        nc.vector.memset(Vext[:, :, :, D:D + 1], 1.0)
        # per-head halves of the attention output (both live on partitions
        # 0-63; the a2a DMA stacks them into the shard's 128 hid rows)
        ATnA = big.tile([64, NG, QB], BF16)
        ATnB = big.tile([64, NG, QB], BF16)

        a2a_in = dram.tile([NG * HIDL, QB], BF16)
        a2a_out = dram.tile([NG * HIDL, QB], BF16)

        # ---- pass 1: K(b0) + Q(b0), chunk-major, overlapped with x^T DMA ----
        streams = [("wk", "bk", KT, rb) for rb in range(NQB)] + \
                  [("wq", "bq", QT, rb) for rb in range(NQB)]
        with tc.tile_pool(name="p1_psum", bufs=1, space="PSUM") as p1:
            p1_tiles = [p1.tile([128, QB], F32, tag=f"p{s}", name=f"p1_{s}")
                        for s in range(8)]
            for i in range(EC):
                for s, (wname, _, _, rb) in enumerate(streams):
                    nc.tensor.matmul(p1_tiles[s][:],
                                     w_tiles[(wname, i)][:],
                                     xT[:, i, QB * rb:QB * (rb + 1)],
                                     start=(i == 0), stop=(i == EC - 1))
            for s, (_, bname, out_t, rb) in enumerate(streams):
                nc.vector.tensor_scalar_add(
                    out=out_t[:, QB * rb:QB * (rb + 1)], in0=p1_tiles[s][:],
                    scalar1=b_tiles[bname][:])

        # ---- attention pools ----
        att_stack = ExitStack()
        att_psum = att_stack.enter_context(
            tc.tile_pool(name="att_psum", bufs=1, space="PSUM"))

        # filler work: each projection stream is split into two half-chunks
        # (4 accumulating matmuls each) so a filler bite never blocks the PE
        # queue for more than ~1us. V streams also emit the DMA-transposes
        # that build Vext (xbar -- no PE or PSUM involvement).
        def proj_stream(wname, bname, out_t, rb):
            state = {}

            def first():
                state["ps"] = att_psum.tile([128, QB], F32, tag="fill",
                                            bufs=2, name="fill_ps")
                for i in range(EC // 2):
                    nc.tensor.matmul(state["ps"][:], w_tiles[(wname, i)][:],
                                     xT[:, i, QB * rb:QB * (rb + 1)],
                                     start=(i == 0), stop=False)

            def second():
                ps = state["ps"]
                for i in range(EC // 2, EC):
                    nc.tensor.matmul(ps[:], w_tiles[(wname, i)][:],
                                     xT[:, i, QB * rb:QB * (rb + 1)],
                                     start=False, stop=(i == EC - 1))
                nc.vector.tensor_scalar_add(
                    out=out_t[:, QB * rb:QB * (rb + 1)], in0=ps[:],
                    scalar1=b_tiles[bname][:])
                if wname == "wv":
                    for kt in range(4 * rb, 4 * rb + 4):
                        stg = rp.tile([128, 128], BF16, tag="vtr", bufs=2,
                                      name="vtr_stg")
                        nc.scalar.dma_start_transpose(
                            stg[:], VT[:, 128 * kt:128 * (kt + 1)])
                        for hl in range(HL):
                            nc.vector.tensor_copy(
                                out=Vext[:, hl, kt, 0:D],
                                in_=stg[:, D * hl:D * (hl + 1)])

            return [first, second]

        def streams_of(specs):
            out = []
            for spec in specs:
                out.extend(proj_stream(*spec))
            return out

        fillers = {
            0: streams_of([("wv", "bv", VT, rb) for rb in range(NQB)]),
            1: streams_of([("wk", "bk", KT, rb)
                           for rb in range(NQB, NQB + 3)]),
            2: streams_of([("wk", "bk", KT, NQB + 3),
                           ("wq", "bq", QT, NQB),
                           ("wq", "bq", QT, NQB + 1)]),
            3: streams_of([("wq", "bq", QT, NQB + 2),
                           ("wq", "bq", QT, NQB + 3),
                           ("wv", "bv", VT, NQB)]),
            4: streams_of([("wv", "bv", VT, rb)
                           for rb in (NQB + 1, NQB + 2, NQB + 3)]),
        }

        # ---- attention: ACT-paced pipeline over 8 (b, qb) units ----
        # pvts[h] rows 0-63 = V.T@E, row 64 = softmax denominator. The norm
        # is split: the PSUM->SBUF eviction runs at the next unit's kc==0
        # (so the pv slots recycle immediately), the reciprocal runs async on
        # DVE, and only a cheap bf16 1-row broadcast matmul + multiply touch
        # the PE stream at kc==6, when the reciprocal is long done.
        def norm_copy(pvts):
            sbs = []
            for h in range(HL):
                sb_pv = rp.tile([D, QB], F32, tag="sbpv", bufs=2,
                                name="sb_pv")
                nc.vector.tensor_copy(out=sb_pv[:], in_=pvts[h][0:D, :])
                # stage the denominator row to partition 0: the custom-DVE
                # reciprocal mis-addresses partition-offset slices
                den = rp.tile([1, QB], F32, tag="den", bufs=2, name="den")
                nc.vector.tensor_copy(out=den[:], in_=pvts[h][D:D + 1, :])
                r = rp.tile([1, QB], F32, tag="r", bufs=2, name="r_row")
                nc.vector.reciprocal_approx_fast(r[:], den[:])
                rb = rp.tile([1, QB], BF16, tag="rb", bufs=2, name="rb_row")
                nc.vector.tensor_copy(out=rb[:], in_=r[:])
                sbs.append((sb_pv, rb))
            return sbs

        def norm_finish(sbs, g):
            for h, atn in ((0, ATnA), (1, ATnB)):
                sb_pv, rb = sbs[h]
                bc = att_psum.tile([128, QB], F32, tag="fill", bufs=2,
                                   name="bc_ps")
                nc.tensor.matmul(bc[0:64, :], ones_t[0:1, :], rb[0:1, :],
                                 start=True, stop=True)
                nc.vector.tensor_mul(out=atn[:, g, :], in0=sb_pv[:],
                                     in1=bc[0:64, :])
            nc.sync.dma_start(out=a2a_in[HIDL * g:HIDL * g + 64, :],
                              in_=ATnA[:, g, :])
            nc.sync.dma_start(out=a2a_in[HIDL * g + 64:HIDL * (g + 1), :],
                              in_=ATnB[:, g, :])

        # PV trails its exp by PV_LAG periods (exp-gated matmuls never block
        # the scores stream); the last PV_LAG chunks of each unit carry over
        # into the next unit's first periods, so unit boundaries have no PV
        # tail in front of the next scores. Norm for the previous unit:
        # PSUM->SBUF copies at p4 (frees the pv slots for this unit's first
        # PV write, also at p4), cheap finish at p8 once the async
        # reciprocal is done.
        PV_LAG = 4
        units = [(b, qb) for b in range(B) for qb in range(NQB)]
        pending_norm = None   # (pvts|sbs, g)
        carry = []            # PV closures carried from the previous unit
        for u, (b, qb) in enumerate(units):
            q0 = b * S + QB * qb
            ebs = []
            fill = list(fillers.get(u, []))
            pvts = [att_psum.tile([128, QB], F32, tag="pv", bufs=2,
                                  name=f"pv{h}") for h in range(HL)]

            def pv_kc(kc, b=b, pvts=pvts, ebs=ebs):
                for h in range(HL):
                    nc.tensor.matmul(
                        pvts[h][0:D + 1, :],
                        Vext[:, h, b * S128 + kc, :],
                        ebs[kc][:, h, :],
                        start=(kc == 0), stop=(kc == S128 - 1))

            for kc in range(S128):
                ps = att_psum.tile([128, HL, QB], F32, tag="sc", bufs=2,
                                   name="sc_ps")
                for h in range(HL):
                    hs = slice(64 * h, 64 * (h + 1))
                    nc.tensor.matmul(
                        ps[:, h, :],
                        KT[hs, b * S + 128 * kc:b * S + 128 * (kc + 1)],
                        QT[hs, q0:q0 + QB], start=True, stop=True)
                eb = ebp.tile([128, HL, QB], BF16, tag="eb", bufs=10,
                              name="eb")
                nc.scalar.activation(eb[:], ps[:], AF.Exp, scale=0.125)
                ebs.append(eb)
                if kc < len(carry):
                    carry[kc]()
                if kc == PV_LAG:
                    carry = []
                    if pending_norm is not None:
                        # copies must precede this unit's first PV write,
                        # which recycles the pv slots the norm still reads
                        pending_norm = (norm_copy(pending_norm[0]),
                                        pending_norm[1])
                if kc >= PV_LAG:
                    pv_kc(kc - PV_LAG)
                if kc == 8 and pending_norm is not None:
                    norm_finish(*pending_norm)
                    pending_norm = None
                if fill:
                    fill.pop(0)()
            carry = [(lambda kc=kc, f=pv_kc: f(kc))
                     for kc in range(S128 - PV_LAG, S128)]
            pending_norm = (pvts, u)

        # drain the final unit
        for c in carry:
            c()
        norm_finish(norm_copy(pending_norm[0]), pending_norm[1])

        nc.gpsimd.collective_compute(
            "AllToAll", mybir.AluOpType.bypass,
            replica_groups=[list(range(N_CORES))],
            ins=[a2a_in.opt()], outs=[a2a_out.opt()])

        # keep the PE clock-gate warm across the AllToAll window; the moving
        # operand is copied from the last attention output so these matmuls
        # are scheduled after attention, inside the A2A wait
        wup_sink = dram.tile([1, 4], BF16)
        wup_sb = stage.tile([1, 4], BF16, tag="wup")
        wup_mv = const.tile([128, 128], BF16)
        nc.vector.memset(wup_mv[:], 1.0)
        nc.vector.tensor_copy(out=wup_mv[0:64, :], in_=ATnB[:, NG - 1, 0:128])
        for _ in range(40):
            wps = att_psum.tile([128, QB], F32, tag="fill", bufs=2,
                                name="wup_ps")
            for w in range(8):
                nc.tensor.matmul(wps[:, 0:128], ident[:], wup_mv[:],
                                 start=(w == 0), stop=(w == 7))
            nc.vector.tensor_copy(out=wup_sb[:], in_=wps[0:1, 0:4])
        nc.sync.dma_start(out=wup_sink[:], in_=wup_sb[:])
        att_stack.close()

        # ---- out projection (transposed: out[e, q] = wo.T-chunks @ A) ----
        AT = big.tile([128, EC, QB], BF16)
        for i in range(EC):
            eng = nc.scalar if i % 2 == 0 else nc.sync
            eng.dma_start(out=AT[:, i, :],
                          in_=a2a_out[HIDL * i:HIDL * (i + 1), :])
        with tc.tile_pool(name="op_psum", bufs=1, space="PSUM") as opp:
            pso = opp.tile([128, EC, QB], F32, name="pso")
            for i in range(EC):
                for e in range(EC):
                    nc.tensor.matmul(pso[:, e, :],
                                     wo_tiles[i][:, 128 * e:128 * (e + 1)],
                                     AT[:, i, :],
                                     start=(i == 0), stop=(i == EC - 1))
            for e in range(EC):
                o_sb = stage.tile([128, QB], F32, tag="osb", bufs=2)
                nc.vector.tensor_scalar_add(out=o_sb[:], in0=pso[:, e, :],
                                            scalar1=bo_sb[:, e:e + 1])
                eng = nc.scalar if e % 2 == 0 else nc.sync
                eng.dma_start(out=out_d[128 * e:128 * (e + 1), :], in_=o_sb[:])

    nc.compile()
    return nc


def shard_inputs(x, Wq, bq, Wk, bk, Wv, bv, Wo, bo, N_CORES=8):
    """Host-side sharding: full fp32 inputs -> per-core in_maps."""
    import ml_dtypes
    bf16 = ml_dtypes.bfloat16
    B, S, E = x.shape
    R = B * S
    HIDL = E // N_CORES
    xt = np.ascontiguousarray(x.reshape(R, E).T).astype(bf16)
    wo = np.ascontiguousarray(Wo).astype(bf16)
    bo_b = np.ascontiguousarray(bo.reshape(E // 128, 128).T).astype(np.float32)
    in_maps = []
    for c in range(N_CORES):
        cs = slice(HIDL * c, HIDL * (c + 1))
        in_maps.append({
            "xt": xt,
            "wq": np.ascontiguousarray(Wq[:, cs]).astype(bf16),
            "wk": np.ascontiguousarray(Wk[:, cs]).astype(bf16),
            "wv": np.ascontiguousarray(Wv[:, cs]).astype(bf16),
            "wo": wo,
            "bq": np.ascontiguousarray(bq[cs].reshape(HIDL, 1)).astype(np.float32),
            "bk": np.ascontiguousarray(bk[cs].reshape(HIDL, 1)).astype(np.float32),
            "bv": np.ascontiguousarray(bv[cs].reshape(HIDL, 1)).astype(np.float32),
            "bo": bo_b,
        })
    return in_maps


def assemble(results, N_CORES=8):
    """Per-core out [E, RL] (core c = q rows 512c..512c+512) -> [B, S, E]."""
    full = np.concatenate([results[i]["out"] for i in range(N_CORES)], axis=1)
    return np.ascontiguousarray(full.T).reshape(2, 2048, 1024)


def kernel(x, Wq, bq, Wk, bk, Wv, bv, Wo, bo):
    from concourse.bass_utils import run_bass_kernel_spmd

    args = [np.asarray(a, dtype=np.float32) for a in
            (x, Wq, bq, Wk, bk, Wv, bv, Wo, bo)]
    if "nc" not in _CACHE:
        _CACHE["nc"] = build_kernel()
    nc = _CACHE["nc"]
    in_maps = shard_inputs(*args)
    res = run_bass_kernel_spmd(nc, in_maps, core_ids=list(range(8)))
    return assemble(res.results)
